# revision 4
# baseline (speedup 1.0000x reference)
"""Data-parallel Trainium kernel for nn_AutoEncoder_16741782520503.

Strategy (per sharding_hint): pure data parallelism over the batch axis.
The 64-image batch is split 8 ways across the 8 NeuronCores; encoder
weights, coords and values are replicated. Each core runs the full
forward pass (encoder MLP -> reparam -> Euler rotation -> bilinear
scatter -> 5x5 Gaussian conv -> rfft2 * CTF -> irfft2) on its 8 images.
Outputs are gathered back to the full [64,128,128,1] array.
"""

import numpy as np

B, XS, P, H = 64, 128, 65536, 1024
N_CORES = 8


def _build_forward():
    import jax
    import jax.numpy as jnp

    def _per_image_standardization(x):
        n = x.shape[1] * x.shape[2] * x.shape[3]
        mean = jnp.mean(x, axis=(1, 2, 3), keepdims=True)
        std = jnp.std(x, axis=(1, 2, 3), keepdims=True)
        return (x - mean) / jnp.maximum(std, 1.0 / np.sqrt(n))

    def _bn(v, p):
        return (v - p[2]) * p[0] / jnp.sqrt(p[3] + 1e-3) + p[1]

    def _euler_rows(rot, tilt, psi):
        ca, sa = jnp.cos(rot), jnp.sin(rot)
        cb, sb = jnp.cos(tilt), jnp.sin(tilt)
        cg, sg = jnp.cos(psi), jnp.sin(psi)
        cc, cs = cb * ca, cb * sa
        row0 = jnp.stack([cg * cc - sg * sa, cg * cs + sg * ca, -cg * sb], axis=-1)
        row1 = jnp.stack([-sg * cc - cg * sa, -sg * cs + cg * ca, sg * sb], axis=-1)
        return row0, row1

    def _gauss_kernel(size=5, sigma=1.0):
        ax = np.arange(size) - (size - 1) / 2.0
        g = np.exp(-(ax ** 2) / (2.0 * sigma ** 2))
        k = np.outer(g, g)
        k /= k.sum()
        return jnp.asarray(k, jnp.float32).reshape(size, size, 1, 1)

    def _forward(images, eps_algn, eps_shift, W0, b0, W_hid, b_hid, Wy, by,
                 Wz, bz, bn_y, bn_z, W_algn, b_algn, W_shift, b_shift,
                 coords, values, rot_batch, tilt_batch, psi_batch, ctf):
        b = images.shape[0]
        xs = images.shape[1]
        x = _per_image_standardization(images)
        h = x.reshape(b, -1)
        h = jax.nn.relu(h @ W0 + b0)
        for i in range(W_hid.shape[0]):
            h = jax.nn.relu(h @ W_hid[i] + b_hid[i])
        y = _bn(jax.nn.relu(h @ Wy + by), bn_y)
        zb = _bn(jax.nn.relu(h @ Wz + bz), bn_z)
        algn = y @ W_algn + b_algn
        shf = zb @ W_shift + b_shift
        mean_a, logvar_a = algn[:, :3], algn[:, 3:]
        mean_s, logvar_s = shf[:, :2], shf[:, 2:]
        z_a = eps_algn * jnp.exp(0.5 * logvar_a) + mean_a
        z_s = eps_shift * jnp.exp(0.5 * logvar_s) + mean_s
        row0, row1 = _euler_rows(rot_batch + z_a[:, 0], tilt_batch + z_a[:, 1],
                                 psi_batch + z_a[:, 2])
        c_x = jnp.einsum('pc,bc->bp', coords, row0) + z_s[:, 0:1]
        c_y = jnp.einsum('pc,bc->bp', coords, row1) + z_s[:, 1:2]
        xmid = 0.5 * xs
        px, py = c_x + xmid, c_y + xmid          # [b, P]
        # Bilinear scatter-add == A^T @ B with separable hat weights.
        # hat(t) = max(0, 1-|t|) at clipped centers reproduces the reference's
        # index clipping exactly (out-of-range taps collapse onto the border).
        pxc = jnp.clip(px, 0.0, xs - 1.0)
        pyc = jnp.clip(py, 0.0, xs - 1.0)
        grid = jnp.arange(xs, dtype=jnp.float32)

        def splat_one(pxc_i, pyc_i):
            Ay = jnp.maximum(0.0, 1.0 - jnp.abs(pyc_i[:, None] - grid[None, :]))
            Ay = Ay * values[:, None]            # [P, 128] weights folded into y
            Bx = jnp.maximum(0.0, 1.0 - jnp.abs(pxc_i[:, None] - grid[None, :]))
            return Ay.T @ Bx                     # [128, 128]

        imgs = jax.vmap(splat_one)(pxc, pyc)     # [b, 128, 128]
        # 5x5 separable Gaussian, SAME zero padding, as shift-and-add
        # (conv_general_dilated hits a broken TransformConvOp pass here).
        ax = np.arange(5) - 2.0
        g = np.exp(-(ax ** 2) / 2.0)
        g = (g / g.sum()).astype(np.float32)
        pad = jnp.pad(imgs, ((0, 0), (2, 2), (0, 0)))
        imgs = sum(g[i] * pad[:, i:i + xs, :] for i in range(5))
        pad = jnp.pad(imgs, ((0, 0), (0, 0), (2, 2)))
        imgs = sum(g[i] * pad[:, :, i:i + xs] for i in range(5))
        imgs = imgs.reshape(b, xs, xs, 1)
        # rfft2 * ctf -> irfft2, expressed as dense 128-point DFT matmuls
        # (jnp.fft is not supported by the neuron compiler).
        nidx = np.arange(xs)
        k65 = np.arange(xs // 2 + 1)
        thx = 2.0 * np.pi * np.outer(nidx, k65) / xs          # [128, 65]
        thy = 2.0 * np.pi * np.outer(nidx, nidx) / xs         # [128, 128]
        Cx = jnp.asarray(np.cos(thx), jnp.float32)
        Sx = jnp.asarray(np.sin(thx), jnp.float32)
        Cy = jnp.asarray(np.cos(thy), jnp.float32)
        Sy = jnp.asarray(np.sin(thy), jnp.float32)
        wk = np.ones(xs // 2 + 1, np.float32) * 2.0
        wk[0] = 1.0
        wk[-1] = 1.0
        Wc = jnp.asarray(Cx * wk[None, :] / xs, jnp.float32)  # [128, 65]
        Ws = jnp.asarray(Sx * wk[None, :] / xs, jnp.float32)
        x2 = imgs[..., 0]                                     # [b, y, n]
        # rfft along last axis
        A = jnp.einsum('byn,nk->byk', x2, Cx)
        Bm = -jnp.einsum('byn,nk->byk', x2, Sx)
        # full fft along y axis
        Fr = jnp.einsum('yq,byk->bqk', Cy, A) + jnp.einsum('yq,byk->bqk', Sy, Bm)
        Fi = jnp.einsum('yq,byk->bqk', Cy, Bm) - jnp.einsum('yq,byk->bqk', Sy, A)
        Gr = Fr * ctf
        Gi = Fi * ctf
        # inverse fft along y axis (1/N)
        ar = (jnp.einsum('qy,bqk->byk', Cy, Gr)
              - jnp.einsum('qy,bqk->byk', Sy, Gi)) / xs
        ai = (jnp.einsum('qy,bqk->byk', Cy, Gi)
              + jnp.einsum('qy,bqk->byk', Sy, Gr)) / xs
        # irfft along last axis with hermitian fold weights
        out = jnp.einsum('byk,nk->byn', ar, Wc) - jnp.einsum('byk,nk->byn', ai, Ws)
        return out[..., None]

    return jax, jnp, _forward


_CACHE = {}


def kernel(**inputs):
    import jax
    jax, jnp, _forward = _build_forward()

    devs = jax.devices()[:N_CORES]
    n = len(devs)

    # Batch-sharded args get a leading device axis; everything else is
    # replicated per core.
    batch_keys = ('images', 'eps_algn', 'eps_shift', 'rot_batch',
                  'tilt_batch', 'psi_batch', 'ctf')
    order = ('images', 'eps_algn', 'eps_shift', 'W0', 'b0', 'W_hid', 'b_hid',
             'Wy', 'by', 'Wz', 'bz', 'bn_y', 'bn_z', 'W_algn', 'b_algn',
             'W_shift', 'b_shift', 'coords', 'values', 'rot_batch',
             'tilt_batch', 'psi_batch', 'ctf')

    if 'pf' not in _CACHE:
        in_axes = tuple(0 if k in batch_keys else None for k in order)
        _CACHE['pf'] = jax.pmap(
            lambda *a: _forward(*a), axis_name='i', in_axes=in_axes,
            devices=devs)

    args = []
    for k in order:
        v = np.asarray(inputs[k])
        if k in batch_keys:
            v = v.reshape((n, v.shape[0] // n) + v.shape[1:])
        args.append(v)

    out = _CACHE['pf'](*args)
    out = np.asarray(out)
    return out.reshape((B,) + out.shape[2:]).astype(np.float32)


if __name__ == '__main__':
    import reference
    inp = reference.setup_inputs()
    inp = {k: np.asarray(v) for k, v in inp.items()}
    exp = np.asarray(reference.reference(**inp))
    act = kernel(**inp)
    err = np.abs(act - exp).max() / (np.abs(exp).max() + 1e-12)
    print('Relative error:', err)
